# revision 20
# baseline (speedup 1.0000x reference)
"""Multi-head self-attention (b=8, t=2048, d=512, 8 heads x dk=64) on 8
Trainium2 NeuronCores, optimized for the axon dispatch-bound regime.

Empirical cost model for this environment (measured via probes): each
instruction costs ~40-90us of dispatch regardless of data size; DMA adds
~1.7us/KB; sem-waits add extra dispatch cost. So the kernel minimizes
instruction count, wait count, and DMA bytes (~1080 instructions/iter and
4MB vs ~2630 and 7MB for the previous data-parallel kernel; measured
3-4.5x faster end to end):

  - Host permutes tokens unmasked-first (pi) and ships x^T directly, so the
    kernel needs NO PE transposes and NO separate gathered-kv tensor:
    xkvT == xT[:, :t_kv]. Output is produced transposed (outT) and the host
    un-permutes/transposes it back. The V bias is folded into bout on the
    host (bout_eff = bv @ Wout + bout; exact because rows of softmax sum
    to 1), and the K bias is kept (the Q-side constant cancels in softmax).
  - Maximal instruction shapes: 512-col matmuls (PSUM bank limit), one
    [128,2048] exp activation per (head, kv-tile). One flat contiguous
    input DMA and one output DMA, at stream boundaries only -- host
    pre-arranges both layouts. (Probed: DMAs issued mid-stream, whether
    split output stores or cross-iteration prefetch, stall the dispatcher
    and cost ~2x; wider-than-512 matmul outputs and 2-wait instructions
    are rejected by walrus; fp8/DoubleRow is numerically unsafe here.)
  - Matmuls ordered so consecutive ones share stationary weights; a post
    pass (a) deletes InstLdweights made redundant by PE weight retention,
    (b) converts non-accumulating matmuls to self-loading (ldweights=True)
    — fusing mid-group corrupts accumulation and fusing group-openers hits
    a slow path, so those keep standalone loads — and (c) drops sem waits
    an instruction holds on its own in-order engine's monotonic counter.
  - Softmax denominator rides as a 65th row through the ctx matmul (ones
    column in V'); normalization = K=1 broadcast matmul + reciprocal + one
    tensor_tensor per head, straight out of PSUM.
"""

import sys
from contextlib import ExitStack

if "/opt/trn_rl_repo" not in sys.path:
    sys.path.insert(0, "/opt/trn_rl_repo")

import numpy as np
import ml_dtypes

import concourse.bass as bass
import concourse.mybir as mybir
import concourse.tile as tile

BF16 = ml_dtypes.bfloat16
T, D = 2048, 512
NH, DK = 8, 64
N_CORES = 8
NEG_BIG = -1.0e30

f32 = mybir.dt.float32
bf16 = mybir.dt.bfloat16

MAX_WAITS = 1


def _split_excess_waits(nc, max_waits=MAX_WAITS):
    """Walrus in this container rejects instructions carrying more than
    ~2 sem-waits. Move the excess onto same-engine nops inserted just before
    the overloaded instruction (engine program order makes this equivalent:
    the engine blocks until every wait is observed either way)."""
    for f in nc.m.functions:
        for bb in f.blocks:
            out = []
            for inst in bb.instructions:
                si = getattr(inst, "sync_info", None)
                if si is not None and si.on_wait and len(si.on_wait) > max_waits:
                    waits = list(si.on_wait)
                    excess, keep = waits[:-max_waits], waits[-max_waits:]
                    si.on_wait = keep
                    for group in range(0, len(excess), max_waits):
                        nop = mybir.InstNoOp(
                            name=f"I-waitsplit-{nc.next_id()}",
                            engine=inst.engine,
                            ins=[],
                            outs=[],
                            sync_info=mybir.SyncInfo(
                                on_wait=excess[group : group + max_waits],
                                on_update=[],
                            ),
                        )
                        out.append(nop)
                out.append(inst)
            bb.instructions[:] = out


def _selfload_safe(mm):
    """Self-loading is fused for matmuls that OPEN an accumulation group
    (start=True; probed correct for both stop values). Mid-group
    (start=False) self-loads corrupt the accumulation and keep their
    standalone InstLdweights."""
    return bool(mm.start_tensor_calc)


def _drop_redundant_waits(nc):
    """Per engine, drop sem-ge waits made redundant by an earlier wait of the
    same engine on the same semaphore with an equal-or-higher target. Engine
    queues execute in order and the counters are monotonic, so an observed
    `S >= v` holds for every later instruction of that engine. Barrier sems
    (sem-eq) are excluded. Tracking resets at basic-block boundaries."""
    n_dropped = 0
    for f in nc.m.functions:
        for bb in f.blocks:
            seen: dict = {}
            for inst in bb.instructions:
                si = getattr(inst, "sync_info", None)
                if si is None or not si.on_wait:
                    continue
                keep = []
                for w in si.on_wait:
                    nm = w.ant_name or ""
                    if w.wait_mode == "sem-ge-imm" and "barrier" not in nm:
                        key = (str(inst.engine), w.id)
                        if seen.get(key, -1) >= w.wait_value:
                            n_dropped += 1
                            continue
                        seen[key] = w.wait_value
                    keep.append(w)
                si.on_wait = keep
    return n_dropped


def _selfload_matmuls(nc, pred=None):
    """Fuse InstLdweights into the following InstMatmult by setting
    ldweights=True and dropping the standalone load (walrus accepts
    self-loading bf16 matmuls; the weight load rides inside the matmul
    instruction, saving a dispatch slot). `pred(matmul_inst)` limits fusion
    to safe matmuls; unsafe ones keep their standalone load."""
    n_fused = 0
    for f in nc.m.functions:
        for bb in f.blocks:
            out = []
            pend = None  # pending InstLdweights awaiting its matmul
            for inst in bb.instructions:
                nm = type(inst).__name__
                if nm == "InstLdweights":
                    assert pend is None, "ldweights not followed by matmul"
                    pend = inst
                    continue
                if nm == "InstMatmult" and pend is not None:
                    if pred is None or pred(inst):
                        inst.ldweights = True
                        psi = pend.sync_info
                        si = inst.sync_info
                        if psi is not None and (psi.on_wait or psi.on_update):
                            if si is None:
                                inst.sync_info = psi
                            else:
                                si.on_wait = list(psi.on_wait) + list(si.on_wait)
                                si.on_update = list(psi.on_update) + list(si.on_update)
                        n_fused += 1
                    else:
                        out.append(pend)
                    pend = None
                out.append(inst)
            assert pend is None, "dangling ldweights at block end"
            bb.instructions[:] = out
    return n_fused


_SELF_SEM_PREFIX = {
    "EngineType.PE": "PE",
    "EngineType.Activation": "Activation",
    "EngineType.DVE": "DVE",
    "EngineType.Pool": "Pool",
}


def _drop_self_waits(nc):
    """Drop sem-ge waits an instruction holds on its OWN engine's completion
    counter. Compute engines execute their queue strictly in order (an
    instruction retires - bumping its sem - before the next is popped), so a
    wait on the engine's own monotonic counter is always already satisfied.
    Each engine's counter is updated exclusively by that engine (verified on
    the built module). DMA queues are excluded (multi-queue, not ordered)."""
    n_dropped = 0
    for f in nc.m.functions:
        for bb in f.blocks:
            for inst in bb.instructions:
                si = getattr(inst, "sync_info", None)
                if si is None or not si.on_wait:
                    continue
                pfx = _SELF_SEM_PREFIX.get(str(inst.engine))
                if pfx is None:
                    continue
                keep = []
                for w in si.on_wait:
                    nm = (w.ant_name or "").rsplit("_", 1)[0]
                    if nm == pfx and w.wait_mode == "sem-ge-imm":
                        n_dropped += 1
                        continue
                    keep.append(w)
                si.on_wait = keep
    return n_dropped


def _ldw_sig(inst):
    a = inst.ins[0]
    return (
        a.memref, a.offset, str(a.ap), str(a.dtype),
        str(inst.perf_mode), str(inst.is_transpose),
        str(inst.tile_position), str(inst.tile_size),
    )


def _dedup_ldweights(nc):
    """Drop InstLdweights identical to the previous PE weight load (the PE
    array retains stationary weights across matmuls). Sem waits/updates of a
    dropped load migrate to the next kept PE instruction."""
    n_removed = 0
    for f in nc.m.functions:
        for bb in f.blocks:
            out = []
            last_sig = None
            pend_waits, pend_updates = [], []
            for inst in bb.instructions:
                nm = type(inst).__name__
                if nm == "InstLdweights":
                    sig = _ldw_sig(inst)
                    if sig == last_sig:
                        si = inst.sync_info
                        if si is not None:
                            pend_waits.extend(si.on_wait)
                            pend_updates.extend(si.on_update)
                        n_removed += 1
                        continue
                    last_sig = sig
                elif nm == "InstMatmult":
                    pass  # matmuls don't clobber PE weights
                elif nm in ("InstTensorCopy", "InstActivation", "InstMemset",
                            "InstTensorTensor", "InstTensorScalarPtr",
                            "InstTensorReduce", "InstDMACopy", "InstReciprocal",
                            "InstNoOp", "InstEventSemaphore", "InstIota",
                            "InstTensorScalarAffineSelect"):
                    pass  # non-PE: weights unaffected
                else:
                    last_sig = None  # unknown: be conservative
                if (pend_waits or pend_updates) and nm in ("InstLdweights", "InstMatmult"):
                    si = inst.sync_info
                    if si is None:
                        inst.sync_info = mybir.SyncInfo(
                            on_wait=list(pend_waits), on_update=list(pend_updates))
                    else:
                        si.on_wait = list(si.on_wait) + pend_waits
                        si.on_update = list(si.on_update) + pend_updates
                    pend_waits, pend_updates = [], []
                out.append(inst)
            assert not pend_waits and not pend_updates, "dangling sem moves"
            bb.instructions[:] = out
    return n_removed


def build_nc(t_kv: int, n_iters: int = 1) -> bass.Bass:
    """Build the per-core kernel. t_kv = padded unmasked-key count (mult of
    128). n_iters > 1 repeats the whole body for timing."""
    nkv = t_kv // 128
    nc = bass.Bass()

    # host pre-arranges x^T so this is a flat contiguous [128, 4*T] copy
    # (xTarr[p, d*T+c] = xT[d*128+p, c]) -- no 3D access pattern needed
    xT_h = nc.declare_dram_parameter("xT", [128, 4 * T], bf16, isOutput=False)
    biasm_h = nc.declare_dram_parameter("bias_m", [128, nkv], f32, isOutput=False)
    wqkv_h = nc.declare_dram_parameter("wqkv", [D, 3 * D], bf16, isOutput=False)
    bq_h = nc.declare_dram_parameter("bq", [128, 4], f32, isOutput=False)
    bk_h = nc.declare_dram_parameter("bk", [128, 4], f32, isOutput=False)
    wout_h = nc.declare_dram_parameter("wout", [D, D], bf16, isOutput=False)
    boutT_h = nc.declare_dram_parameter("boutT", [128, 4], f32, isOutput=False)
    outT_h = nc.declare_dram_parameter("outT", [128, 4 * T], bf16, isOutput=True)

    Exp = mybir.ActivationFunctionType.Exp
    add_op = mybir.AluOpType.add
    mult_op = mybir.AluOpType.mult

    with tile.TileContext(nc) as tc, ExitStack() as ctx:
        cpool = ctx.enter_context(tc.tile_pool(name="const", bufs=1))

        ones_row = cpool.tile([1, 128], bf16, tag="ones")
        nc.vector.memset(ones_row[:], 1.0)
        wqkv_sb = []
        for k in range(4):
            w = cpool.tile([128, 3 * D], bf16, tag=f"wqkv{k}", name=f"wqkv{k}")
            nc.sync.dma_start(out=w[:], in_=wqkv_h[k * 128 : (k + 1) * 128, :])
            wqkv_sb.append(w)
        wout_sb = []
        for k in range(4):
            w = cpool.tile([128, D], bf16, tag=f"wout{k}", name=f"wout{k}")
            nc.sync.dma_start(out=w[:], in_=wout_h[k * 128 : (k + 1) * 128, :])
            wout_sb.append(w)
        bq_sb = cpool.tile([128, 4], f32, tag="bq")
        nc.sync.dma_start(out=bq_sb[:], in_=bq_h[:])
        bk_sb = cpool.tile([128, 4], f32, tag="bk")
        nc.sync.dma_start(out=bk_sb[:], in_=bk_h[:])
        boutT_sb = cpool.tile([128, 4], f32, tag="boutT")
        nc.sync.dma_start(out=boutT_sb[:], in_=boutT_h[:])

        pers = ctx.enter_context(tc.tile_pool(name="pers", bufs=1))
        ptp = ctx.enter_context(tc.tile_pool(name="ptp", bufs=2))
        # PSUM: slot A [128,2048] f32 = 4 banks, slot B [65|128,2048] = 4 banks
        psA = ctx.enter_context(tc.tile_pool(name="psA", bufs=1, space="PSUM"))
        psB = ctx.enter_context(tc.tile_pool(name="psB", bufs=1, space="PSUM"))

        env = dict(t_kv=t_kv, nkv=nkv, ones_row=ones_row, wqkv_sb=wqkv_sb,
                   wout_sb=wout_sb, bq_sb=bq_sb, bk_sb=bk_sb,
                   boutT_sb=boutT_sb, xT_h=xT_h, biasm_h=biasm_h, outT_h=outT_h,
                   pers=pers, ptp=ptp, psA=psA, psB=psB)

        # NOTE: dynamic For_i loops wedge the device on this exec path
        # (BSP dispatch does not support branching) -- unroll instead.
        for _ in range(n_iters):
            _body(tc, nc, env, Exp, add_op, mult_op)

    return nc


def _body(tc, nc, env, Exp, add_op, mult_op):
    t_kv, nkv = env["t_kv"], env["nkv"]
    ones_row = env["ones_row"]
    wqkv_sb, wout_sb = env["wqkv_sb"], env["wout_sb"]
    bq_sb, bk_sb, boutT_sb = env["bq_sb"], env["bk_sb"], env["boutT_sb"]
    xT_h, biasm_h, outT_h = env["xT_h"], env["biasm_h"], env["outT_h"]
    pers, ptp, psA, psB = env["pers"], env["ptp"], env["psA"], env["psB"]

    if True:
        kstride = -(-t_kv // 512) * 512  # k-block column stride, bank-aligned
        xTall = pers.tile([128, 4 * T], bf16, tag="xTall", name="xTall")
        qTall = pers.tile([128, 4 * T], bf16, tag="qTall", name="qTall")
        kTall = pers.tile([128, 4 * kstride], bf16, tag="kTall", name="kTall")
        vp = pers.tile([128, nkv * 520], bf16, tag="vp", name="vp")
        ctxTall = pers.tile([128, 4 * T], bf16, tag="ctxTall", name="ctxTall")
        otall = pers.tile([128, 4 * T], bf16, tag="otall", name="otall")
        dsb = pers.tile([1, T], bf16, tag="dsb", name="dsb")
        rden64 = pers.tile([64, T], bf16, tag="rden64", name="rden64")
        bias_sb = pers.tile([128, nkv], f32, tag="biasm")

        # ---- loads: flat contiguous DMAs (host pre-arranged layouts) ----
        nc.sync.dma_start(out=xTall[:], in_=xT_h[:])
        nc.sync.dma_start(out=bias_sb[:], in_=biasm_h[:])
        nc.gpsimd.memset(vp[:], 1.0)  # ones column 64 of each head block

        # ---- Q projection: qT[m] = (Wq[:,m].T @ xT) + bq, stationary reused
        # across the 4 q-column chunks ----
        for m in range(4):
            pq = psA.tile([128, T], f32, tag="A", name="pq")
            for k in range(4):
                for c in range(4):
                    nc.tensor.matmul(
                        pq[:, c * 512 : (c + 1) * 512],
                        wqkv_sb[k][:, m * 128 : (m + 1) * 128],
                        xTall[:, k * T + c * 512 : k * T + (c + 1) * 512],
                        start=(k == 0), stop=(k == 3),
                    )
            nc.vector.tensor_scalar(
                qTall[:, m * T : (m + 1) * T], pq[:], bq_sb[:, m : m + 1],
                None, add_op,
            )

        # ---- K projection over the first t_kv (unmasked-first) tokens ----
        kchunks = []
        off = 0
        while off < t_kv:
            cl = min(512, t_kv - off)
            kchunks.append((off, cl))
            off += cl
        for m in range(4):
            pk = psA.tile([128, T], f32, tag="A", name="pk")
            for k in range(4):
                for (o, cl) in kchunks:
                    nc.tensor.matmul(
                        pk[:, o : o + cl],
                        wqkv_sb[k][:, 512 + m * 128 : 512 + (m + 1) * 128],
                        xTall[:, k * T + o : k * T + o + cl],
                        start=(k == 0), stop=(k == 3),
                    )
            nc.vector.tensor_scalar(
                kTall[:, m * kstride : m * kstride + t_kv], pk[:, :t_kv],
                bk_sb[:, m : m + 1], None, add_op,
            )

        # ---- V projection, batched 4 kv-tiles per PSUM slot. The V bias is
        # folded into bout on the host (bout_eff = bv @ Wout + bout): after
        # softmax-normalization the bv term adds exactly bv to every ctx row.
        mt = 0
        while mt < nkv:
            g = min(4, nkv - mt)
            pv = psB.tile([128, 2048], f32, tag="B", name="pv")
            for k in range(4):
                for i in range(g):
                    nc.tensor.matmul(
                        pv[:, i * 512 : (i + 1) * 512],
                        xTall[:, k * T + (mt + i) * 128 : k * T + (mt + i + 1) * 128],
                        wqkv_sb[k][:, 1024:1536],
                        start=(k == 0), stop=(k == 3),
                    )
            dst = vp[:, mt * 520 : (mt + g) * 520]
            dst4 = dst.rearrange("p (g h c) -> p g h c", h=NH, c=65)[:, :, :, 0:64]
            src4 = pv[:, : g * 512].rearrange("p (g h c) -> p g h c", g=g, c=64)
            nc.vector.tensor_copy(dst4, src4)
            mt += g

        # ---- attention + fused normalization, head by head ----
        for h in range(NH):
            m = h // 2
            p0 = (h % 2) * 64
            prow = slice(p0, p0 + 64)
            pctx = psB.tile([65, T], f32, tag="B", name="pctx")
            for kvi in range(nkv):
                ps = psA.tile([128, T], f32, tag="A", name="ps")
                for c in range(4):
                    nc.tensor.matmul(
                        ps[:, c * 512 : (c + 1) * 512],
                        kTall[prow, m * kstride + kvi * 128 : m * kstride + (kvi + 1) * 128],
                        qTall[prow, m * T + c * 512 : m * T + (c + 1) * 512],
                        start=True, stop=True,
                    )
                pt = ptp.tile([128, T], bf16, tag="pt", name="pt")
                nc.scalar.activation(
                    pt[:], ps[:], Exp,
                    bias=bias_sb[:, kvi : kvi + 1], scale=0.125,
                )
                for c in range(4):
                    nc.tensor.matmul(
                        pctx[:, c * 512 : (c + 1) * 512],
                        vp[:, kvi * 520 + h * 65 : kvi * 520 + (h + 1) * 65],
                        pt[:, c * 512 : (c + 1) * 512],
                        start=(kvi == 0), stop=(kvi == nkv - 1),
                    )
            # normalization: denom row 64 -> SBUF, K=1 broadcast to 64
            # partitions, reciprocal, multiply straight out of PSUM
            nc.vector.tensor_copy(dsb[:], pctx[64:65, :])
            pbc = psA.tile([128, T], f32, tag="A", name="pbc")
            for c in range(4):
                nc.tensor.matmul(
                    pbc[0:64, c * 512 : (c + 1) * 512],
                    ones_row[:1, :64], dsb[:1, c * 512 : (c + 1) * 512],
                    start=True, stop=True,
                )
            with nc.allow_low_precision("softmax denom recip in bf16"):
                nc.vector.reciprocal(rden64[:], pbc[0:64, :])
            nc.vector.tensor_tensor(
                ctxTall[prow, m * T : (m + 1) * T],
                pctx[0:64, :], rden64[:], mult_op,
            )

        # ---- output projection: outT[m] = Wout[:,m].T @ ctxT + bout ----
        for m in range(4):
            po = psA.tile([128, T], f32, tag="A", name="po")
            for k in range(4):
                for c in range(4):
                    nc.tensor.matmul(
                        po[:, c * 512 : (c + 1) * 512],
                        wout_sb[k][:, m * 128 : (m + 1) * 128],
                        ctxTall[:, k * T + c * 512 : k * T + (c + 1) * 512],
                        start=(k == 0), stop=(k == 3),
                    )
            nc.vector.tensor_scalar(
                otall[:, m * T : (m + 1) * T], po[:], boutT_sb[:, m : m + 1],
                None, add_op,
            )

        nc.sync.dma_start(
            out=outT_h[:],
            in_=otall[:],
        )


_NC_CACHE: dict = {}


def _get_nc(t_kv: int, n_iters: int = 1, split_waits: bool = True) -> bass.Bass:
    key = (t_kv, n_iters)
    if key not in _NC_CACHE:
        nc = build_nc(t_kv, n_iters)
        _dedup_ldweights(nc)
        # Self-loading is only safe for non-accumulating matmuls (walrus
        # mis-encodes ldweights=True inside start=False/stop=False groups);
        # retained weights then serve the rest of the dedup'd run.
        _selfload_matmuls(nc, pred=_selfload_safe)
        _drop_self_waits(nc)
        _drop_redundant_waits(nc)
        _NC_CACHE[key] = [nc, False]
    ent = _NC_CACHE[key]
    if split_waits and not ent[1]:
        _split_excess_waits(ent[0])
        ent[1] = True
    return ent[0]


def make_in_maps(x, mask, Wqkv, bqkv, Wout, bout, t_kv: int):
    nkv = t_kv // 128
    bout_eff = bqkv[1024:1536].astype(np.float64) @ Wout.astype(np.float64) + bout
    shared = {
        "wqkv": np.ascontiguousarray(Wqkv).astype(BF16),
        "wout": np.ascontiguousarray(Wout).astype(BF16),
        "bq": np.ascontiguousarray(bqkv[0:512].reshape(4, 128).T).astype(np.float32),
        "bk": np.ascontiguousarray(bqkv[512:1024].reshape(4, 128).T).astype(np.float32),
        "boutT": np.ascontiguousarray(bout_eff.reshape(4, 128).T).astype(np.float32),
    }
    in_maps, perms = [], []
    for b in range(N_CORES):
        mrow = np.asarray(mask[b, 0]) != 0
        perm = np.concatenate([np.nonzero(mrow)[0], np.nonzero(~mrow)[0]])
        cnt = int(mrow.sum())
        # [512, 2048] -> [128, 4*2048] with block d at cols d*T (flat DMA)
        xT = np.ascontiguousarray(
            x[b][perm].T.reshape(4, 128, T).transpose(1, 0, 2).reshape(128, 4 * T)
        ).astype(BF16)
        biasvec = np.where(np.arange(t_kv) < cnt, 0.0, NEG_BIG).astype(np.float32)
        bias_m = np.ascontiguousarray(biasvec.reshape(nkv, 128).T)
        in_maps.append({**shared, "xT": xT, "bias_m": bias_m})
        perms.append(perm)
    return in_maps, perms


def pick_t_kv(mask) -> int:
    counts = (np.asarray(mask)[:, 0, :] != 0).sum(axis=1)
    return max(128, int(-(-int(counts.max()) // 128)) * 128)


def kernel(x, mask, Wqkv, bqkv, Wout, bout):
    from concourse.bass_utils import run_bass_kernel_spmd

    x = np.asarray(x, dtype=np.float32)
    mask = np.asarray(mask)
    Wqkv = np.asarray(Wqkv, dtype=np.float32)
    bqkv = np.asarray(bqkv, dtype=np.float32)
    Wout = np.asarray(Wout, dtype=np.float32)
    bout = np.asarray(bout, dtype=np.float32)

    t_kv = pick_t_kv(mask)
    nc = _get_nc(t_kv)
    in_maps, perms = make_in_maps(x, mask, Wqkv, bqkv, Wout, bout, t_kv)
    res = run_bass_kernel_spmd(nc, in_maps, list(range(N_CORES)))
    out = np.empty((N_CORES, T, D), dtype=np.float32)
    for b in range(N_CORES):
        oT = np.asarray(res.results[b]["outT"], dtype=np.float32)
        oT = oT.reshape(128, 4, T).transpose(1, 0, 2).reshape(D, T)
        out[b, perms[b], :] = oT.T
    return out


# revision 21
# speedup vs baseline: 1.4229x; 1.4229x over previous
"""Multi-head self-attention (b=8, t=2048, d=512, 8 heads x dk=64) on 8
Trainium2 NeuronCores, optimized for the axon dispatch-bound regime.

Empirical cost model for this environment (measured via probes): each
instruction costs ~40-90us of dispatch regardless of data size; DMA adds
~1.7us/KB; sem-waits add extra dispatch cost. So the kernel minimizes
instruction count, wait count, and DMA bytes (~1080 instructions/iter and
4MB vs ~2630 and 7MB for the previous data-parallel kernel; measured
3-4.5x faster end to end):

  - Host permutes tokens unmasked-first (pi) and ships x^T directly, so the
    kernel needs NO PE transposes and NO separate gathered-kv tensor:
    xkvT == xT[:, :t_kv]. Output is produced transposed (outT) and the host
    un-permutes/transposes it back. The V bias is folded into bout on the
    host (bout_eff = bv @ Wout + bout; exact because rows of softmax sum
    to 1), and the K bias is kept (the Q-side constant cancels in softmax).
  - Maximal instruction shapes: 512-col matmuls (PSUM bank limit), one
    [128,2048] exp activation per (head, kv-tile). One flat contiguous
    input DMA and one output DMA, at stream boundaries only -- host
    pre-arranges both layouts. (Probed: DMAs issued mid-stream, whether
    split output stores or cross-iteration prefetch, stall the dispatcher
    and cost ~2x; wider-than-512 matmul outputs and 2-wait instructions
    are rejected by walrus; fp8/DoubleRow is numerically unsafe here.)
  - Matmuls ordered so consecutive ones share stationary weights; a post
    pass (a) deletes InstLdweights made redundant by PE weight retention,
    (b) converts non-accumulating matmuls to self-loading (ldweights=True)
    — fusing mid-group corrupts accumulation and fusing group-openers hits
    a slow path, so those keep standalone loads — and (c) drops sem waits
    an instruction holds on its own in-order engine's monotonic counter.
  - Softmax denominator rides as a 65th row through the ctx matmul (ones
    column in V'); normalization = K=1 broadcast matmul + reciprocal + one
    tensor_tensor per head, straight out of PSUM.
"""

import sys
from contextlib import ExitStack

if "/opt/trn_rl_repo" not in sys.path:
    sys.path.insert(0, "/opt/trn_rl_repo")

import numpy as np
import ml_dtypes

import concourse.bass as bass
import concourse.mybir as mybir
import concourse.tile as tile

BF16 = ml_dtypes.bfloat16
T, D = 2048, 512
NH, DK = 8, 64
N_CORES = 8
NEG_BIG = -1.0e30

f32 = mybir.dt.float32
bf16 = mybir.dt.bfloat16

MAX_WAITS = 1


def _split_excess_waits(nc, max_waits=MAX_WAITS):
    """Walrus in this container rejects instructions carrying more than
    ~2 sem-waits. Move the excess onto same-engine nops inserted just before
    the overloaded instruction (engine program order makes this equivalent:
    the engine blocks until every wait is observed either way)."""
    for f in nc.m.functions:
        for bb in f.blocks:
            out = []
            for inst in bb.instructions:
                si = getattr(inst, "sync_info", None)
                if si is not None and si.on_wait and len(si.on_wait) > max_waits:
                    waits = list(si.on_wait)
                    excess, keep = waits[:-max_waits], waits[-max_waits:]
                    si.on_wait = keep
                    for group in range(0, len(excess), max_waits):
                        nop = mybir.InstNoOp(
                            name=f"I-waitsplit-{nc.next_id()}",
                            engine=inst.engine,
                            ins=[],
                            outs=[],
                            sync_info=mybir.SyncInfo(
                                on_wait=excess[group : group + max_waits],
                                on_update=[],
                            ),
                        )
                        out.append(nop)
                out.append(inst)
            bb.instructions[:] = out


def _selfload_safe(mm):
    """Self-loading is only fused for non-accumulating matmuls (start & stop
    both True). Mid-group (start=False) self-loads corrupt the accumulation
    (probed), and group-opening (start=True, stop=False) self-loads are
    functionally correct but ~0.7ms/instruction SLOWER than a standalone
    load (measured twice: +10ms and +20ms for 29 fusions), so both keep
    their InstLdweights."""
    return bool(mm.start_tensor_calc) and bool(mm.stop_tensor_calc)


def _drop_redundant_waits(nc):
    """Per engine, drop sem-ge waits made redundant by an earlier wait of the
    same engine on the same semaphore with an equal-or-higher target. Engine
    queues execute in order and the counters are monotonic, so an observed
    `S >= v` holds for every later instruction of that engine. Barrier sems
    (sem-eq) are excluded. Tracking resets at basic-block boundaries."""
    n_dropped = 0
    for f in nc.m.functions:
        for bb in f.blocks:
            seen: dict = {}
            for inst in bb.instructions:
                si = getattr(inst, "sync_info", None)
                if si is None or not si.on_wait:
                    continue
                keep = []
                for w in si.on_wait:
                    nm = w.ant_name or ""
                    if w.wait_mode == "sem-ge-imm" and "barrier" not in nm:
                        key = (str(inst.engine), w.id)
                        if seen.get(key, -1) >= w.wait_value:
                            n_dropped += 1
                            continue
                        seen[key] = w.wait_value
                    keep.append(w)
                si.on_wait = keep
    return n_dropped


def _selfload_matmuls(nc, pred=None):
    """Fuse InstLdweights into the following InstMatmult by setting
    ldweights=True and dropping the standalone load (walrus accepts
    self-loading bf16 matmuls; the weight load rides inside the matmul
    instruction, saving a dispatch slot). `pred(matmul_inst)` limits fusion
    to safe matmuls; unsafe ones keep their standalone load."""
    n_fused = 0
    for f in nc.m.functions:
        for bb in f.blocks:
            out = []
            pend = None  # pending InstLdweights awaiting its matmul
            for inst in bb.instructions:
                nm = type(inst).__name__
                if nm == "InstLdweights":
                    assert pend is None, "ldweights not followed by matmul"
                    pend = inst
                    continue
                if nm == "InstMatmult" and pend is not None:
                    if pred is None or pred(inst):
                        inst.ldweights = True
                        psi = pend.sync_info
                        si = inst.sync_info
                        if psi is not None and (psi.on_wait or psi.on_update):
                            if si is None:
                                inst.sync_info = psi
                            else:
                                si.on_wait = list(psi.on_wait) + list(si.on_wait)
                                si.on_update = list(psi.on_update) + list(si.on_update)
                        n_fused += 1
                    else:
                        out.append(pend)
                    pend = None
                out.append(inst)
            assert pend is None, "dangling ldweights at block end"
            bb.instructions[:] = out
    return n_fused


_SELF_SEM_PREFIX = {
    "EngineType.PE": "PE",
    "EngineType.Activation": "Activation",
    "EngineType.DVE": "DVE",
    "EngineType.Pool": "Pool",
}


def _drop_self_waits(nc):
    """Drop sem-ge waits an instruction holds on its OWN engine's completion
    counter. Compute engines execute their queue strictly in order (an
    instruction retires - bumping its sem - before the next is popped), so a
    wait on the engine's own monotonic counter is always already satisfied.
    Each engine's counter is updated exclusively by that engine (verified on
    the built module). DMA queues are excluded (multi-queue, not ordered)."""
    n_dropped = 0
    for f in nc.m.functions:
        for bb in f.blocks:
            for inst in bb.instructions:
                si = getattr(inst, "sync_info", None)
                if si is None or not si.on_wait:
                    continue
                pfx = _SELF_SEM_PREFIX.get(str(inst.engine))
                if pfx is None:
                    continue
                keep = []
                for w in si.on_wait:
                    nm = (w.ant_name or "").rsplit("_", 1)[0]
                    if nm == pfx and w.wait_mode == "sem-ge-imm":
                        n_dropped += 1
                        continue
                    keep.append(w)
                si.on_wait = keep
    return n_dropped


def _ldw_sig(inst):
    a = inst.ins[0]
    return (
        a.memref, a.offset, str(a.ap), str(a.dtype),
        str(inst.perf_mode), str(inst.is_transpose),
        str(inst.tile_position), str(inst.tile_size),
    )


def _dedup_ldweights(nc):
    """Drop InstLdweights identical to the previous PE weight load (the PE
    array retains stationary weights across matmuls). Sem waits/updates of a
    dropped load migrate to the next kept PE instruction."""
    n_removed = 0
    for f in nc.m.functions:
        for bb in f.blocks:
            out = []
            last_sig = None
            pend_waits, pend_updates = [], []
            for inst in bb.instructions:
                nm = type(inst).__name__
                if nm == "InstLdweights":
                    sig = _ldw_sig(inst)
                    if sig == last_sig:
                        si = inst.sync_info
                        if si is not None:
                            pend_waits.extend(si.on_wait)
                            pend_updates.extend(si.on_update)
                        n_removed += 1
                        continue
                    last_sig = sig
                elif nm == "InstMatmult":
                    pass  # matmuls don't clobber PE weights
                elif nm in ("InstTensorCopy", "InstActivation", "InstMemset",
                            "InstTensorTensor", "InstTensorScalarPtr",
                            "InstTensorReduce", "InstDMACopy", "InstReciprocal",
                            "InstNoOp", "InstEventSemaphore", "InstIota",
                            "InstTensorScalarAffineSelect"):
                    pass  # non-PE: weights unaffected
                else:
                    last_sig = None  # unknown: be conservative
                if (pend_waits or pend_updates) and nm in ("InstLdweights", "InstMatmult"):
                    si = inst.sync_info
                    if si is None:
                        inst.sync_info = mybir.SyncInfo(
                            on_wait=list(pend_waits), on_update=list(pend_updates))
                    else:
                        si.on_wait = list(si.on_wait) + pend_waits
                        si.on_update = list(si.on_update) + pend_updates
                    pend_waits, pend_updates = [], []
                out.append(inst)
            assert not pend_waits and not pend_updates, "dangling sem moves"
            bb.instructions[:] = out
    return n_removed


def build_nc(t_kv: int, n_iters: int = 1) -> bass.Bass:
    """Build the per-core kernel. t_kv = padded unmasked-key count (mult of
    128). n_iters > 1 repeats the whole body for timing."""
    nkv = t_kv // 128
    nc = bass.Bass()

    # host pre-arranges x^T so this is a flat contiguous [128, 4*T] copy
    # (xTarr[p, d*T+c] = xT[d*128+p, c]) -- no 3D access pattern needed
    xT_h = nc.declare_dram_parameter("xT", [128, 4 * T], bf16, isOutput=False)
    biasm_h = nc.declare_dram_parameter("bias_m", [128, nkv], f32, isOutput=False)
    wqkv_h = nc.declare_dram_parameter("wqkv", [D, 3 * D], bf16, isOutput=False)
    bq_h = nc.declare_dram_parameter("bq", [128, 4], f32, isOutput=False)
    bk_h = nc.declare_dram_parameter("bk", [128, 4], f32, isOutput=False)
    wout_h = nc.declare_dram_parameter("wout", [D, D], bf16, isOutput=False)
    boutT_h = nc.declare_dram_parameter("boutT", [128, 4], f32, isOutput=False)
    outT_h = nc.declare_dram_parameter("outT", [128, 4 * T], bf16, isOutput=True)

    Exp = mybir.ActivationFunctionType.Exp
    add_op = mybir.AluOpType.add
    mult_op = mybir.AluOpType.mult

    with tile.TileContext(nc) as tc, ExitStack() as ctx:
        cpool = ctx.enter_context(tc.tile_pool(name="const", bufs=1))

        ones_row = cpool.tile([1, 128], bf16, tag="ones")
        nc.vector.memset(ones_row[:], 1.0)
        wqkv_sb = []
        for k in range(4):
            w = cpool.tile([128, 3 * D], bf16, tag=f"wqkv{k}", name=f"wqkv{k}")
            nc.sync.dma_start(out=w[:], in_=wqkv_h[k * 128 : (k + 1) * 128, :])
            wqkv_sb.append(w)
        wout_sb = []
        for k in range(4):
            w = cpool.tile([128, D], bf16, tag=f"wout{k}", name=f"wout{k}")
            nc.sync.dma_start(out=w[:], in_=wout_h[k * 128 : (k + 1) * 128, :])
            wout_sb.append(w)
        bq_sb = cpool.tile([128, 4], f32, tag="bq")
        nc.sync.dma_start(out=bq_sb[:], in_=bq_h[:])
        bk_sb = cpool.tile([128, 4], f32, tag="bk")
        nc.sync.dma_start(out=bk_sb[:], in_=bk_h[:])
        boutT_sb = cpool.tile([128, 4], f32, tag="boutT")
        nc.sync.dma_start(out=boutT_sb[:], in_=boutT_h[:])

        pers = ctx.enter_context(tc.tile_pool(name="pers", bufs=1))
        ptp = ctx.enter_context(tc.tile_pool(name="ptp", bufs=2))
        # PSUM: slot A [128,2048] f32 = 4 banks, slot B [65|128,2048] = 4 banks
        psA = ctx.enter_context(tc.tile_pool(name="psA", bufs=1, space="PSUM"))
        psB = ctx.enter_context(tc.tile_pool(name="psB", bufs=1, space="PSUM"))

        env = dict(t_kv=t_kv, nkv=nkv, ones_row=ones_row, wqkv_sb=wqkv_sb,
                   wout_sb=wout_sb, bq_sb=bq_sb, bk_sb=bk_sb,
                   boutT_sb=boutT_sb, xT_h=xT_h, biasm_h=biasm_h, outT_h=outT_h,
                   pers=pers, ptp=ptp, psA=psA, psB=psB)

        # NOTE: dynamic For_i loops wedge the device on this exec path
        # (BSP dispatch does not support branching) -- unroll instead.
        for _ in range(n_iters):
            _body(tc, nc, env, Exp, add_op, mult_op)

    return nc


def _body(tc, nc, env, Exp, add_op, mult_op):
    t_kv, nkv = env["t_kv"], env["nkv"]
    ones_row = env["ones_row"]
    wqkv_sb, wout_sb = env["wqkv_sb"], env["wout_sb"]
    bq_sb, bk_sb, boutT_sb = env["bq_sb"], env["bk_sb"], env["boutT_sb"]
    xT_h, biasm_h, outT_h = env["xT_h"], env["biasm_h"], env["outT_h"]
    pers, ptp, psA, psB = env["pers"], env["ptp"], env["psA"], env["psB"]

    if True:
        kstride = -(-t_kv // 512) * 512  # k-block column stride, bank-aligned
        xTall = pers.tile([128, 4 * T], bf16, tag="xTall", name="xTall")
        qTall = pers.tile([128, 4 * T], bf16, tag="qTall", name="qTall")
        kTall = pers.tile([128, 4 * kstride], bf16, tag="kTall", name="kTall")
        vp = pers.tile([128, nkv * 520], bf16, tag="vp", name="vp")
        ctxTall = pers.tile([128, 4 * T], bf16, tag="ctxTall", name="ctxTall")
        otall = pers.tile([128, 4 * T], bf16, tag="otall", name="otall")
        dsb = pers.tile([1, T], bf16, tag="dsb", name="dsb")
        rden64 = pers.tile([64, T], bf16, tag="rden64", name="rden64")
        bias_sb = pers.tile([128, nkv], f32, tag="biasm")

        # ---- loads: flat contiguous DMAs (host pre-arranged layouts) ----
        nc.sync.dma_start(out=xTall[:], in_=xT_h[:])
        nc.sync.dma_start(out=bias_sb[:], in_=biasm_h[:])
        nc.gpsimd.memset(vp[:], 1.0)  # ones column 64 of each head block

        # ---- Q projection: qT[m] = (Wq[:,m].T @ xT) + bq, stationary reused
        # across the 4 q-column chunks ----
        for m in range(4):
            pq = psA.tile([128, T], f32, tag="A", name="pq")
            for k in range(4):
                for c in range(4):
                    nc.tensor.matmul(
                        pq[:, c * 512 : (c + 1) * 512],
                        wqkv_sb[k][:, m * 128 : (m + 1) * 128],
                        xTall[:, k * T + c * 512 : k * T + (c + 1) * 512],
                        start=(k == 0), stop=(k == 3),
                    )
            nc.vector.tensor_scalar(
                qTall[:, m * T : (m + 1) * T], pq[:], bq_sb[:, m : m + 1],
                None, add_op,
            )

        # ---- K projection over the first t_kv (unmasked-first) tokens ----
        kchunks = []
        off = 0
        while off < t_kv:
            cl = min(512, t_kv - off)
            kchunks.append((off, cl))
            off += cl
        for m in range(4):
            pk = psA.tile([128, T], f32, tag="A", name="pk")
            for k in range(4):
                for (o, cl) in kchunks:
                    nc.tensor.matmul(
                        pk[:, o : o + cl],
                        wqkv_sb[k][:, 512 + m * 128 : 512 + (m + 1) * 128],
                        xTall[:, k * T + o : k * T + o + cl],
                        start=(k == 0), stop=(k == 3),
                    )
            nc.vector.tensor_scalar(
                kTall[:, m * kstride : m * kstride + t_kv], pk[:, :t_kv],
                bk_sb[:, m : m + 1], None, add_op,
            )

        # ---- V projection, batched 4 kv-tiles per PSUM slot. The V bias is
        # folded into bout on the host (bout_eff = bv @ Wout + bout): after
        # softmax-normalization the bv term adds exactly bv to every ctx row.
        mt = 0
        while mt < nkv:
            g = min(4, nkv - mt)
            pv = psB.tile([128, 2048], f32, tag="B", name="pv")
            for k in range(4):
                for i in range(g):
                    nc.tensor.matmul(
                        pv[:, i * 512 : (i + 1) * 512],
                        xTall[:, k * T + (mt + i) * 128 : k * T + (mt + i + 1) * 128],
                        wqkv_sb[k][:, 1024:1536],
                        start=(k == 0), stop=(k == 3),
                    )
            dst = vp[:, mt * 520 : (mt + g) * 520]
            dst4 = dst.rearrange("p (g h c) -> p g h c", h=NH, c=65)[:, :, :, 0:64]
            src4 = pv[:, : g * 512].rearrange("p (g h c) -> p g h c", g=g, c=64)
            nc.vector.tensor_copy(dst4, src4)
            mt += g

        # ---- attention + fused normalization, head by head ----
        for h in range(NH):
            m = h // 2
            p0 = (h % 2) * 64
            prow = slice(p0, p0 + 64)
            pctx = psB.tile([65, T], f32, tag="B", name="pctx")
            for kvi in range(nkv):
                ps = psA.tile([128, T], f32, tag="A", name="ps")
                for c in range(4):
                    nc.tensor.matmul(
                        ps[:, c * 512 : (c + 1) * 512],
                        kTall[prow, m * kstride + kvi * 128 : m * kstride + (kvi + 1) * 128],
                        qTall[prow, m * T + c * 512 : m * T + (c + 1) * 512],
                        start=True, stop=True,
                    )
                pt = ptp.tile([128, T], bf16, tag="pt", name="pt")
                nc.scalar.activation(
                    pt[:], ps[:], Exp,
                    bias=bias_sb[:, kvi : kvi + 1], scale=0.125,
                )
                for c in range(4):
                    nc.tensor.matmul(
                        pctx[:, c * 512 : (c + 1) * 512],
                        vp[:, kvi * 520 + h * 65 : kvi * 520 + (h + 1) * 65],
                        pt[:, c * 512 : (c + 1) * 512],
                        start=(kvi == 0), stop=(kvi == nkv - 1),
                    )
            # normalization: denom row 64 -> SBUF, K=1 broadcast to 64
            # partitions, reciprocal, multiply straight out of PSUM
            nc.vector.tensor_copy(dsb[:], pctx[64:65, :])
            pbc = psA.tile([128, T], f32, tag="A", name="pbc")
            for c in range(4):
                nc.tensor.matmul(
                    pbc[0:64, c * 512 : (c + 1) * 512],
                    ones_row[:1, :64], dsb[:1, c * 512 : (c + 1) * 512],
                    start=True, stop=True,
                )
            with nc.allow_low_precision("softmax denom recip in bf16"):
                nc.vector.reciprocal(rden64[:], pbc[0:64, :])
            nc.vector.tensor_tensor(
                ctxTall[prow, m * T : (m + 1) * T],
                pctx[0:64, :], rden64[:], mult_op,
            )

        # ---- output projection: outT[m] = Wout[:,m].T @ ctxT + bout ----
        for m in range(4):
            po = psA.tile([128, T], f32, tag="A", name="po")
            for k in range(4):
                for c in range(4):
                    nc.tensor.matmul(
                        po[:, c * 512 : (c + 1) * 512],
                        wout_sb[k][:, m * 128 : (m + 1) * 128],
                        ctxTall[:, k * T + c * 512 : k * T + (c + 1) * 512],
                        start=(k == 0), stop=(k == 3),
                    )
            nc.vector.tensor_scalar(
                otall[:, m * T : (m + 1) * T], po[:], boutT_sb[:, m : m + 1],
                None, add_op,
            )

        nc.sync.dma_start(
            out=outT_h[:],
            in_=otall[:],
        )


_NC_CACHE: dict = {}


def _get_nc(t_kv: int, n_iters: int = 1, split_waits: bool = True) -> bass.Bass:
    key = (t_kv, n_iters)
    if key not in _NC_CACHE:
        nc = build_nc(t_kv, n_iters)
        _dedup_ldweights(nc)
        # Self-loading is only safe for non-accumulating matmuls (walrus
        # mis-encodes ldweights=True inside start=False/stop=False groups);
        # retained weights then serve the rest of the dedup'd run.
        _selfload_matmuls(nc, pred=_selfload_safe)
        _drop_self_waits(nc)
        _drop_redundant_waits(nc)
        _NC_CACHE[key] = [nc, False]
    ent = _NC_CACHE[key]
    if split_waits and not ent[1]:
        _split_excess_waits(ent[0])
        ent[1] = True
    return ent[0]


def make_in_maps(x, mask, Wqkv, bqkv, Wout, bout, t_kv: int):
    nkv = t_kv // 128
    bout_eff = bqkv[1024:1536].astype(np.float64) @ Wout.astype(np.float64) + bout
    shared = {
        "wqkv": np.ascontiguousarray(Wqkv).astype(BF16),
        "wout": np.ascontiguousarray(Wout).astype(BF16),
        "bq": np.ascontiguousarray(bqkv[0:512].reshape(4, 128).T).astype(np.float32),
        "bk": np.ascontiguousarray(bqkv[512:1024].reshape(4, 128).T).astype(np.float32),
        "boutT": np.ascontiguousarray(bout_eff.reshape(4, 128).T).astype(np.float32),
    }
    in_maps, perms = [], []
    for b in range(N_CORES):
        mrow = np.asarray(mask[b, 0]) != 0
        perm = np.concatenate([np.nonzero(mrow)[0], np.nonzero(~mrow)[0]])
        cnt = int(mrow.sum())
        # [512, 2048] -> [128, 4*2048] with block d at cols d*T (flat DMA)
        xT = np.ascontiguousarray(
            x[b][perm].T.reshape(4, 128, T).transpose(1, 0, 2).reshape(128, 4 * T)
        ).astype(BF16)
        biasvec = np.where(np.arange(t_kv) < cnt, 0.0, NEG_BIG).astype(np.float32)
        bias_m = np.ascontiguousarray(biasvec.reshape(nkv, 128).T)
        in_maps.append({**shared, "xT": xT, "bias_m": bias_m})
        perms.append(perm)
    return in_maps, perms


def pick_t_kv(mask) -> int:
    counts = (np.asarray(mask)[:, 0, :] != 0).sum(axis=1)
    return max(128, int(-(-int(counts.max()) // 128)) * 128)


def kernel(x, mask, Wqkv, bqkv, Wout, bout):
    from concourse.bass_utils import run_bass_kernel_spmd

    x = np.asarray(x, dtype=np.float32)
    mask = np.asarray(mask)
    Wqkv = np.asarray(Wqkv, dtype=np.float32)
    bqkv = np.asarray(bqkv, dtype=np.float32)
    Wout = np.asarray(Wout, dtype=np.float32)
    bout = np.asarray(bout, dtype=np.float32)

    t_kv = pick_t_kv(mask)
    nc = _get_nc(t_kv)
    in_maps, perms = make_in_maps(x, mask, Wqkv, bqkv, Wout, bout, t_kv)
    res = run_bass_kernel_spmd(nc, in_maps, list(range(N_CORES)))
    out = np.empty((N_CORES, T, D), dtype=np.float32)
    for b in range(N_CORES):
        oT = np.asarray(res.results[b]["outT"], dtype=np.float32)
        oT = oT.reshape(128, 4, T).transpose(1, 0, 2).reshape(D, T)
        out[b, perms[b], :] = oT.T
    return out
